# revision 7
# baseline (speedup 1.0000x reference)
"""Trainium2 Bass kernel for brute-force kNN (nn_BruteForce_72541997629642).

Problem: queries [256, 64] f32, candidates [1e6, 64] f32, ids [1e6] i32,
k=10.  reference: scores = queries @ candidates.T; top_k(scores, k).

Strategy (8 NeuronCores, candidates sharded along N, queries replicated):
  Device (per core, 125k candidates, padded to 2*W):
    - host pre-transposes + bf16-casts the candidate shard into
      cT [128, W]: partition rows 0:64 hold dims of shard candidates
      [0, W) ("half A"), rows 64:128 hold dims of candidates [W, 2W)
      ("half B") -> every DMA uses all 128 partitions at full width,
    - TensorE: bf16 matmuls, queries stationary ([64, 128] per query
      group, auto row-tiled at partition base 0/64), candidates moving
      (512 cols per matmul = 1 PSUM bank),
    - VectorE: reduce_max over 32-candidate blocks straight from PSUM
      ([128, 64, 32] -> [128, 64] per 4-bank PSUM tile),
    - optional ScalarE-assisted path (ACT_FRAC of tiles): ACT casts PSUM
      fp32 -> SBUF bf16, DVE folds chunk pairs with 2x-mode bf16
      tensor-tensor max before a bf16 reduce (drains PSUM faster than the
      1x-only tensor_reduce can alone),
    - DMA the per-query block-max table bm [2, 128, NB] to DRAM.
  Host:
    - pick top BLK_TAKE block-entries per (core, query) from bm,
    - gather those blocks' candidate vectors, exact fp32 rescore,
      merge all cores, exact global top-k.

The bf16 matmul only drives block *selection* (top-k blocks by block-max
provably contain the exact top-k candidates; bf16 noise is covered by the
BLK_TAKE margin over k).  All returned scores/indices come from the exact
fp32 host rescore, so indices match the fp32 reference exactly and scores
to ~1e-7 relative.
"""

from contextlib import ExitStack

import ml_dtypes
import numpy as np

import concourse.bass as bass
import concourse.mybir as mybir
import concourse.tile as tile
from concourse.bass_utils import run_bass_kernel_spmd
from concourse.vector_clock import ScopedClock

f32 = mybir.dt.float32
bf16 = mybir.dt.bfloat16

# ---------------- problem constants (hardcoded per spec) ----------------
B = 256          # queries
D = 64           # dims
N = 1_000_000    # candidates
NCORES = 8
NSHARD = N // NCORES          # 125000 candidates per core
W = 63488                     # half-shard width (= 31 * 2048)
NPAD = 2 * W                  # padded per-core candidates (126976)
BS = 32                       # block size for block-max
NB = NPAD // BS               # block slots per core (3968)
F_DMA = 2048                  # candidate columns per DMA tile
F_MM = 512                    # moving free dim per matmul (1 PSUM bank)
N_DMA_TILES = W // F_DMA      # 31
BLK_TAKE = 16                 # block entries taken per (core, query) on host

# (g, h, ti) combos drained via the ScalarE cast + bf16-fold path instead of
# direct DVE reduce_max. h-half 1 -> ACT path when enabled.
ACT_PATH = False

_MAX_WAITS = 1


def _split_excess_waits(nc):
    """Walrus codegen rejects instructions carrying more than ~1 sem-wait
    (varies by ISA struct).  Move excess waits onto same-engine NoOps
    inserted immediately before the offending instruction — the engine
    blocks at the NoOps instead, which is semantically identical."""
    n_nops = 0
    for f in nc.m.functions:
        for bb in f.blocks:
            new_insts = []
            dirty = False
            for ins in bb.instructions:
                si = ins.sync_info
                if (
                    si is not None
                    and si.on_wait is not None
                    and len(si.on_wait) > _MAX_WAITS
                ):
                    waits = list(si.on_wait)
                    keep = waits[: _MAX_WAITS]
                    rest = waits[_MAX_WAITS:]
                    for j in range(0, len(rest), _MAX_WAITS):
                        nop = mybir.InstNoOp(name=f"I-waitsplit-{n_nops}")
                        n_nops += 1
                        nop.engine = ins.engine
                        nop.sync_info = mybir.SyncInfo(
                            on_wait=rest[j : j + _MAX_WAITS], on_update=[]
                        )
                        new_insts.append(nop)
                    ins.sync_info = mybir.SyncInfo(
                        on_wait=keep, on_update=list(si.on_update or [])
                    )
                    dirty = True
                new_insts.append(ins)
            if dirty:
                bb.instructions = new_insts
    return n_nops


def _build_nc(repeat: int = 1, act_path: bool = ACT_PATH, loop_repeat: int = 0):
    """Build the per-core Bass program. See module docstring for layout.

    loop_repeat > 0 wraps the streaming body in a device-side For loop that
    runs it that many times — used only for wall-clock timing of the
    steady-state kernel (amortizes dispatch/transfer overheads).
    """
    nc = bass.Bass()
    qT = nc.dram_tensor("qT", [128, B], bf16, kind="ExternalInput")
    cT = nc.dram_tensor("cT", [128, W], bf16, kind="ExternalInput")
    bm = nc.dram_tensor("bm", [2, 128, NB], f32, kind="ExternalOutput")

    with tile.TileContext(nc) as tc, ExitStack() as ctx:
        qpool = ctx.enter_context(tc.tile_pool(name="qpool", bufs=1))
        cpool = ctx.enter_context(tc.tile_pool(name="cpool", bufs=3))
        pp = ctx.enter_context(tc.tile_pool(name="pp", bufs=2, space="PSUM"))
        bmp = ctx.enter_context(tc.tile_pool(name="bmp", bufs=1))
        sbp = ctx.enter_context(tc.tile_pool(name="sbp", bufs=4))

        qt = qpool.tile([128, B], bf16)
        nc.sync.dma_start(out=qt[:], in_=qT[:])
        bm_sb = [
            bmp.tile([128, NB], f32, name=f"bmsb{g}", tag=f"bmsb{g}") for g in range(2)
        ]

        from contextlib import nullcontext

        def body_ctx():
            if loop_repeat > 0:
                return tc.For_i(
                    0,
                    loop_repeat,
                    1,
                    hint_engines=(
                        mybir.EngineType.PE,
                        mybir.EngineType.DVE,
                        mybir.EngineType.SP,
                        mybir.EngineType.Activation,
                    ),
                )
            return nullcontext()

        with body_ctx():
          for _rep in range(repeat):
            for ti in range(N_DMA_TILES):
                ct = cpool.tile([128, F_DMA], bf16)
                nc.sync.dma_start(
                    out=ct[:], in_=cT[:, ti * F_DMA : (ti + 1) * F_DMA]
                )
                for g in range(2):
                    for h in range(2):
                        boff = (h * W + ti * F_DMA) // BS  # 64 block slots
                        use_act = act_path and h == 1
                        ps = pp.tile([128, F_DMA], f32, name="ps", tag="ps")
                        for sub in range(4):
                            nc.tensor.matmul(
                                out=ps[:, 512 * sub : 512 * (sub + 1)],
                                lhsT=qt[
                                    64 * h : 64 * (h + 1), 128 * g : 128 * (g + 1)
                                ],
                                rhs=ct[
                                    64 * h : 64 * (h + 1), 512 * sub : 512 * (sub + 1)
                                ],
                                start=True,
                                stop=True,
                            )
                        if not use_act:
                            nc.vector.reduce_max(
                                out=bm_sb[g][:, boff : boff + F_DMA // BS],
                                in_=ps[:].rearrange("p (nb bs) -> p nb bs", bs=BS),
                                axis=mybir.AxisListType.X,
                            )
                        else:
                            # ACT drains PSUM (fp32->bf16); DVE folds the two
                            # 1024-chunks, then one more fold + bf16 reduce.
                            # Fold pairs chunk j with chunk j+1024cols: the
                            # 32 reduced blockmaxes each cover 4 blocks:
                            # slots boff+b, +16, +32, +48 share value at all
                            # four (host treats slots as entries; duplicates
                            # merely waste a pick, handled by host dedupe
                            # against entry groups).
                            c0 = sbp.tile([128, 1024], bf16, name="c0", tag="c0")
                            c1 = sbp.tile([128, 1024], bf16, name="c1", tag="c1")
                            nc.scalar.copy(out=c0[:], in_=ps[:, :1024])
                            nc.scalar.copy(out=c1[:], in_=ps[:, 1024:])
                            fold = sbp.tile([128, 1024], bf16, name="fold", tag="fold")
                            nc.vector.tensor_tensor(
                                out=fold[:], in0=c0[:], in1=c1[:],
                                op=mybir.AluOpType.max,
                            )
                            red = sbp.tile([128, 32], f32, name="red", tag="red")
                            nc.vector.reduce_max(
                                out=red[:],
                                in_=fold[:].rearrange("p (nb bs) -> p nb bs", bs=BS),
                                axis=mybir.AxisListType.X,
                            )
                            # entry covers blocks {b, b+32} (fold of 1024-col
                            # pair) -> write value to both slot groups
                            nc.vector.tensor_copy(
                                out=bm_sb[g][:, boff : boff + 32], in_=red[:]
                            )
                            nc.vector.tensor_copy(
                                out=bm_sb[g][:, boff + 32 : boff + 64], in_=red[:]
                            )
            for g in range(2):
                nc.sync.dma_start(out=bm[g], in_=bm_sb[g][:])
    _split_excess_waits(nc)
    nc.finalize()
    return nc


_NC_CACHE: dict[tuple, object] = {}


def get_nc(repeat: int = 1, act_path: bool = ACT_PATH):
    key = (repeat, act_path)
    if key not in _NC_CACHE:
        _NC_CACHE[key] = _build_nc(repeat, act_path)
    return _NC_CACHE[key]


def _prep_inputs(queries: np.ndarray, candidates: np.ndarray):
    q = np.asarray(queries, dtype=np.float32)
    c = np.asarray(candidates, dtype=np.float32)
    qT = np.ascontiguousarray(q.T)  # [64, 256]
    qT2 = np.concatenate([qT, qT], axis=0).astype(ml_dtypes.bfloat16)
    in_maps = []
    for core in range(NCORES):
        shard = c[core * NSHARD : (core + 1) * NSHARD]
        half_a = shard[:W]
        half_b = shard[W:]
        cT2 = np.zeros((128, W), dtype=ml_dtypes.bfloat16)
        cT2[:D, :] = half_a.T.astype(ml_dtypes.bfloat16)
        cT2[D:, : half_b.shape[0]] = half_b.T.astype(ml_dtypes.bfloat16)
        in_maps.append({"qT": qT2, "cT": cT2})
    return in_maps


def _host_finish(bm_all, queries, candidates, ids, k, act_path: bool = ACT_PATH):
    """bm_all: [NCORES, 2, 128, NB] f32 block-max tables -> exact top-k."""
    q = np.asarray(queries, dtype=np.float32)
    c = np.asarray(candidates, dtype=np.float32)
    ids = np.asarray(ids)
    k = int(k)
    bm = bm_all.reshape(NCORES, B, NB).transpose(1, 0, 2).copy()  # [B, 8, NB]

    if act_path:
        # ACT-path tiles wrote each entry value to 4 duplicate slots
        # (blocks b, b+32 within half-B tiles are one entry; plus the
        # mirrored copy). Suppress duplicates so argpartition picks
        # distinct entries: keep only the first slot group per tile.
        for ti in range(N_DMA_TILES):
            boff = (W + ti * F_DMA) // BS
            bm[:, :, boff + 32 : boff + 64] = -np.inf

    take = min(BLK_TAKE, NB)
    part = np.argpartition(-bm, take - 1, axis=2)[:, :, :take]  # [B, 8, T]

    # expand selected entries to candidate blocks
    if act_path:
        # entries within half B cover blocks {b, b+32} of their tile
        hb = part >= (W // BS)
        in_tile_off = np.where(hb, (part - W // BS) % 64, 0)
        partner = np.where(hb & (in_tile_off < 32), part + 32, part)
        blocks = np.concatenate([part, partner], axis=2)  # [B, 8, 2T]
    else:
        blocks = part

    local = blocks[..., None] * BS + np.arange(BS)  # [B, 8, T', 32]
    core_off = (np.arange(NCORES) * NSHARD)[None, :, None, None]
    valid = local < NSHARD
    gidx = np.where(valid, local + core_off, 0)
    Bq = B
    gidx = gidx.reshape(Bq, -1)
    valid = valid.reshape(Bq, -1)

    P = gidx.shape[1]
    top_scores = np.empty((B, k), dtype=np.float32)
    top_idx = np.empty((B, k), dtype=np.int32)
    CH = 64
    for q0 in range(0, B, CH):
        q1 = min(q0 + CH, B)
        gi = gidx[q0:q1]
        gath = c[gi]  # [ch, P, 64]
        sc = np.einsum("qd,qpd->qp", q[q0:q1], gath, optimize=True)
        sc = np.where(valid[q0:q1], sc, -np.inf).astype(np.float32)
        for qq in range(q1 - q0):
            row = sc[qq]
            gx = gi[qq]
            m = min(4 * k, P - 1)
            sel = np.argpartition(-row, m)[: m + 1]
            order = np.lexsort((gx[sel], -row[sel]))
            seen = set()
            out_s, out_i = [], []
            for o in order:
                cid = int(gx[sel[o]])
                s = row[sel[o]]
                if cid in seen or not np.isfinite(s):
                    continue
                seen.add(cid)
                out_s.append(s)
                out_i.append(cid)
                if len(out_s) == k:
                    break
            top_scores[q0 + qq] = out_s
            top_idx[q0 + qq] = ids[np.asarray(out_i, dtype=np.int64)]
    return top_scores, top_idx


def kernel(queries, candidates, ids, k):
    in_maps = _prep_inputs(queries, candidates)
    nc = get_nc(repeat=1, act_path=ACT_PATH)
    res = run_bass_kernel_spmd(nc, in_maps, core_ids=list(range(NCORES)))
    bm_all = np.stack([res.results[c]["bm"] for c in range(NCORES)])
    return _host_finish(
        bm_all,
        np.asarray(queries, np.float32),
        np.asarray(candidates, np.float32),
        np.asarray(ids),
        int(k),
        act_path=ACT_PATH,
    )


# revision 11
# speedup vs baseline: 7.6706x; 7.6706x over previous
"""Trainium2 Bass kernel for brute-force kNN (nn_BruteForce_72541997629642).

Problem: queries [256, 64] f32, candidates [1e6, 64] f32, ids [1e6] i32,
k=10.  reference: scores = queries @ candidates.T; top_k(scores, k).

Strategy (8 NeuronCores, candidates sharded along N, queries replicated):
  Device (per core, 125k candidates, padded to 2*W):
    - host pre-transposes + bf16-casts the candidate shard into
      cT [128, W]: partition rows 0:64 hold dims of shard candidates
      [0, W) ("half A"), rows 64:128 hold dims of candidates [W, 2W)
      ("half B") -> every DMA uses all 128 partitions at full width,
    - TensorE: bf16 matmuls, queries stationary ([64, 128] per query
      group, auto row-tiled at partition base 0/64), candidates moving
      (512 cols per matmul = 1 PSUM bank),
    - VectorE: reduce_max over 32-candidate blocks straight from PSUM
      ([128, 64, 32] -> [128, 64] per 4-bank PSUM tile),
    - optional ScalarE-assisted path (ACT_FRAC of tiles): ACT casts PSUM
      fp32 -> SBUF bf16, DVE folds chunk pairs with 2x-mode bf16
      tensor-tensor max before a bf16 reduce (drains PSUM faster than the
      1x-only tensor_reduce can alone),
    - DMA the per-query block-max table bm [2, 128, NB] to DRAM.
  Host:
    - pick top BLK_TAKE block-entries per (core, query) from bm,
    - gather those blocks' candidate vectors, exact fp32 rescore,
      merge all cores, exact global top-k.

The bf16 matmul only drives block *selection* (top-k blocks by block-max
provably contain the exact top-k candidates; bf16 noise is covered by the
BLK_TAKE margin over k).  All returned scores/indices come from the exact
fp32 host rescore, so indices match the fp32 reference exactly and scores
to ~1e-7 relative.
"""

from contextlib import ExitStack

import ml_dtypes
import numpy as np

import concourse.bass as bass
import concourse.mybir as mybir
import concourse.tile as tile
from concourse.bass_utils import run_bass_kernel_spmd
from concourse.vector_clock import ScopedClock

f32 = mybir.dt.float32
bf16 = mybir.dt.bfloat16

# ---------------- problem constants (hardcoded per spec) ----------------
B = 256          # queries
D = 64           # dims
N = 1_000_000    # candidates
NCORES = 8
NSHARD = N // NCORES          # 125000 candidates per core
W = 63488                     # half-shard width (= 31 * 2048)
NPAD = 2 * W                  # padded per-core candidates (126976)
BS = 32                       # block size for block-max
NB = NPAD // BS               # block slots per core (3968)
F_DMA = 2048                  # candidate columns per DMA tile
F_MM = 512                    # moving free dim per matmul (1 PSUM bank)
N_DMA_TILES = W // F_DMA      # 31
BLK_TAKE = 16                 # block entries taken per (core, query) on host

# ACT-assisted PSUM drain: for tiles with ti % ACT_DIRECT_EVERY != 0, the
# ScalarE casts both halves' PSUM scores to SBUF bf16 and the VectorE folds
# them (2x bf16 tensor-tensor max) before one bf16 reduce — draining PSUM
# ~1.5x faster than the 1x-only fp32 tensor_reduce path can alone. The
# resulting block "entries" each cover 4 blocks (128 candidates); the host
# expands them. Tiles with ti % ACT_DIRECT_EVERY == 0 use the direct DVE
# reduce (keeps the VectorE busy while ScalarE is the unit bottleneck).
ACT_PATH = True
ACT_DIRECT_EVERY = 5

_MAX_WAITS = 1


def _split_excess_waits(nc):
    """Walrus codegen rejects instructions carrying more than ~1 sem-wait
    (varies by ISA struct).  Move excess waits onto same-engine NoOps
    inserted immediately before the offending instruction — the engine
    blocks at the NoOps instead, which is semantically identical."""
    n_nops = 0
    for f in nc.m.functions:
        for bb in f.blocks:
            new_insts = []
            dirty = False
            for ins in bb.instructions:
                si = ins.sync_info
                if (
                    si is not None
                    and si.on_wait is not None
                    and len(si.on_wait) > _MAX_WAITS
                ):
                    waits = list(si.on_wait)
                    keep = waits[: _MAX_WAITS]
                    rest = waits[_MAX_WAITS:]
                    for j in range(0, len(rest), _MAX_WAITS):
                        nop = mybir.InstNoOp(name=f"I-waitsplit-{n_nops}")
                        n_nops += 1
                        nop.engine = ins.engine
                        nop.sync_info = mybir.SyncInfo(
                            on_wait=rest[j : j + _MAX_WAITS], on_update=[]
                        )
                        new_insts.append(nop)
                    ins.sync_info = mybir.SyncInfo(
                        on_wait=keep, on_update=list(si.on_update or [])
                    )
                    dirty = True
                new_insts.append(ins)
            if dirty:
                bb.instructions = new_insts
    return n_nops


def _build_nc(repeat: int = 1, act_path: bool = ACT_PATH, loop_repeat: int = 0):
    """Build the per-core Bass program. See module docstring for layout.

    loop_repeat > 0 wraps the streaming body in a device-side For loop that
    runs it that many times — used only for wall-clock timing of the
    steady-state kernel (amortizes dispatch/transfer overheads).
    """
    nc = bass.Bass()
    qT = nc.dram_tensor("qT", [128, B], bf16, kind="ExternalInput")
    cT = nc.dram_tensor("cT", [128, W], bf16, kind="ExternalInput")
    bm = nc.dram_tensor("bm", [2, 128, NB], f32, kind="ExternalOutput")

    with tile.TileContext(nc) as tc, ExitStack() as ctx:
        qpool = ctx.enter_context(tc.tile_pool(name="qpool", bufs=1))
        cpool = ctx.enter_context(tc.tile_pool(name="cpool", bufs=3))
        pp = ctx.enter_context(tc.tile_pool(name="pp", bufs=2, space="PSUM"))
        bmp = ctx.enter_context(tc.tile_pool(name="bmp", bufs=1))
        sbp = ctx.enter_context(tc.tile_pool(name="sbp", bufs=4))

        qt = qpool.tile([128, B], bf16)
        nc.sync.dma_start(out=qt[:], in_=qT[:])
        bm_sb = [
            bmp.tile([128, NB], f32, name=f"bmsb{g}", tag=f"bmsb{g}") for g in range(2)
        ]
        if act_path:
            # ACT units leave most bm slots unwritten (host masks them);
            # memset once so the output DMA never reads uninitialized SBUF.
            for g in range(2):
                nc.gpsimd.memset(bm_sb[g][:], -3.0e38)

        from contextlib import nullcontext

        def body_ctx():
            if loop_repeat > 0:
                return tc.For_i(
                    0,
                    loop_repeat,
                    1,
                    hint_engines=(
                        mybir.EngineType.PE,
                        mybir.EngineType.DVE,
                        mybir.EngineType.SP,
                        mybir.EngineType.Activation,
                    ),
                )
            return nullcontext()

        with body_ctx():
          for _rep in range(repeat):
            for ti in range(N_DMA_TILES):
                ct = cpool.tile([128, F_DMA], bf16)
                nc.sync.dma_start(
                    out=ct[:], in_=cT[:, ti * F_DMA : (ti + 1) * F_DMA]
                )
                use_act = act_path and (ti % ACT_DIRECT_EVERY != 0)
                for g in range(2):
                    boff_a = (ti * F_DMA) // BS          # half A slots
                    if not use_act:
                        for h in range(2):
                            boff = (h * W + ti * F_DMA) // BS
                            ps = pp.tile([128, F_DMA], f32, name="ps", tag="ps")
                            for sub in range(4):
                                nc.tensor.matmul(
                                    out=ps[:, 512 * sub : 512 * (sub + 1)],
                                    lhsT=qt[
                                        64 * h : 64 * (h + 1), 128 * g : 128 * (g + 1)
                                    ],
                                    rhs=ct[
                                        64 * h : 64 * (h + 1),
                                        512 * sub : 512 * (sub + 1),
                                    ],
                                    start=True,
                                    stop=True,
                                )
                            nc.vector.reduce_max(
                                out=bm_sb[g][:, boff : boff + F_DMA // BS],
                                in_=ps[:].rearrange("p (nb bs) -> p nb bs", bs=BS),
                                axis=mybir.AxisListType.X,
                            )
                    else:
                        # ACT unit: both halves' scores (4096 candidates).
                        # ACT casts each PSUM mega-tile to SBUF bf16; DVE:
                        # f1 = max(castA, castB)  [128, 2048]  (bf16 2x)
                        # f2 = max(f1[:, :1024], f1[:, 1024:])
                        # red = blockmax(f2)      [128, 32]
                        # entry b covers blocks {A: b, b+32, B: b, b+32}.
                        casts = []
                        for h in range(2):
                            ps = pp.tile([128, F_DMA], f32, name="ps", tag="ps")
                            for sub in range(4):
                                nc.tensor.matmul(
                                    out=ps[:, 512 * sub : 512 * (sub + 1)],
                                    lhsT=qt[
                                        64 * h : 64 * (h + 1), 128 * g : 128 * (g + 1)
                                    ],
                                    rhs=ct[
                                        64 * h : 64 * (h + 1),
                                        512 * sub : 512 * (sub + 1),
                                    ],
                                    start=True,
                                    stop=True,
                                )
                            cst = sbp.tile(
                                [128, F_DMA], bf16, name=f"cast{h}", tag=f"cast{h}"
                            )
                            nc.scalar.copy(out=cst[:], in_=ps[:])
                            casts.append(cst)
                        f1 = sbp.tile([128, F_DMA], bf16, name="f1", tag="f1")
                        nc.vector.tensor_tensor(
                            out=f1[:], in0=casts[0][:], in1=casts[1][:],
                            op=mybir.AluOpType.max,
                        )
                        f2 = sbp.tile([128, F_DMA // 2], bf16, name="f2", tag="f2")
                        nc.vector.tensor_tensor(
                            out=f2[:], in0=f1[:, : F_DMA // 2], in1=f1[:, F_DMA // 2 :],
                            op=mybir.AluOpType.max,
                        )
                        nc.vector.reduce_max(
                            out=bm_sb[g][:, boff_a : boff_a + 32],
                            in_=f2[:].rearrange("p (nb bs) -> p nb bs", bs=BS),
                            axis=mybir.AxisListType.X,
                        )
            for g in range(2):
                nc.sync.dma_start(out=bm[g], in_=bm_sb[g][:])
    _split_excess_waits(nc)
    nc.finalize()
    return nc


_NC_CACHE: dict[tuple, object] = {}


def get_nc(repeat: int = 1, act_path: bool = ACT_PATH):
    key = (repeat, act_path)
    if key not in _NC_CACHE:
        _NC_CACHE[key] = _build_nc(repeat, act_path)
    return _NC_CACHE[key]


def _prep_inputs(queries: np.ndarray, candidates: np.ndarray):
    q = np.asarray(queries, dtype=np.float32)
    c = np.asarray(candidates, dtype=np.float32)
    qT = np.ascontiguousarray(q.T)  # [64, 256]
    qT2 = np.concatenate([qT, qT], axis=0).astype(ml_dtypes.bfloat16)
    in_maps = []
    for core in range(NCORES):
        shard = c[core * NSHARD : (core + 1) * NSHARD]
        half_a = shard[:W]
        half_b = shard[W:]
        cT2 = np.zeros((128, W), dtype=ml_dtypes.bfloat16)
        cT2[:D, :] = half_a.T.astype(ml_dtypes.bfloat16)
        cT2[D:, : half_b.shape[0]] = half_b.T.astype(ml_dtypes.bfloat16)
        in_maps.append({"qT": qT2, "cT": cT2})
    return in_maps


def _host_finish(bm_all, queries, candidates, ids, k, act_path: bool = ACT_PATH):
    """bm_all: [NCORES, 2, 128, NB] f32 block-max tables -> exact top-k."""
    q = np.asarray(queries, dtype=np.float32)
    c = np.asarray(candidates, dtype=np.float32)
    ids = np.asarray(ids)
    k = int(k)
    bm = bm_all.reshape(NCORES, B, NB).transpose(1, 0, 2).copy()  # [B, 8, NB]

    NBH = W // BS  # slots per half
    if act_path:
        # ACT units only write the first 32 slots of their tile's half-A
        # range; every other slot of an ACT tile is dead (-3e38 from the
        # device memset). Mask defensively, then expand selected entry
        # slots to their 4 covered blocks.
        slots = np.arange(NB)
        in_half_a = slots < NBH
        ti_of = np.where(in_half_a, slots // 64, (slots - NBH) // 64)
        is_act = (ti_of % ACT_DIRECT_EVERY) != 0
        b_in_tile = np.where(in_half_a, slots % 64, (slots - NBH) % 64)
        entry_slot = in_half_a & is_act & (b_in_tile < 32)
        dead = is_act & ~entry_slot
        bm[:, :, dead] = -np.inf

    take = min(BLK_TAKE, NB)
    part = np.argpartition(-bm, take - 1, axis=2)[:, :, :take]  # [B, 8, T]

    if act_path:
        sel_entry = entry_slot[part]  # [B, 8, T]
        partners = np.stack(
            [part, part + 32, part + NBH, part + NBH + 32], axis=-1
        )  # [B, 8, T, 4]
        blocks = np.where(sel_entry[..., None], partners, part[..., None])
        blocks = blocks.reshape(B, NCORES, -1)
    else:
        blocks = part

    local = blocks[..., None] * BS + np.arange(BS)  # [B, 8, T', 32]
    core_off = (np.arange(NCORES) * NSHARD)[None, :, None, None]
    valid = local < NSHARD
    gidx = np.where(valid, local + core_off, 0)
    Bq = B
    gidx = gidx.reshape(Bq, -1)
    valid = valid.reshape(Bq, -1)

    P = gidx.shape[1]
    top_scores = np.empty((B, k), dtype=np.float32)
    top_idx = np.empty((B, k), dtype=np.int32)
    CH = 64
    for q0 in range(0, B, CH):
        q1 = min(q0 + CH, B)
        gi = gidx[q0:q1]
        gath = c[gi]  # [ch, P, 64]
        sc = np.einsum("qd,qpd->qp", q[q0:q1], gath, optimize=True)
        sc = np.where(valid[q0:q1], sc, -np.inf).astype(np.float32)
        for qq in range(q1 - q0):
            row = sc[qq]
            gx = gi[qq]
            m = min(4 * k, P - 1)
            sel = np.argpartition(-row, m)[: m + 1]
            order = np.lexsort((gx[sel], -row[sel]))
            seen = set()
            out_s, out_i = [], []
            for o in order:
                cid = int(gx[sel[o]])
                s = row[sel[o]]
                if cid in seen or not np.isfinite(s):
                    continue
                seen.add(cid)
                out_s.append(s)
                out_i.append(cid)
                if len(out_s) == k:
                    break
            top_scores[q0 + qq] = out_s
            top_idx[q0 + qq] = ids[np.asarray(out_i, dtype=np.int64)]
    return top_scores, top_idx


def kernel(queries, candidates, ids, k):
    in_maps = _prep_inputs(queries, candidates)
    nc = get_nc(repeat=1, act_path=ACT_PATH)
    res = run_bass_kernel_spmd(nc, in_maps, core_ids=list(range(NCORES)))
    bm_all = np.stack([res.results[c]["bm"] for c in range(NCORES)])
    return _host_finish(
        bm_all,
        np.asarray(queries, np.float32),
        np.asarray(candidates, np.float32),
        np.asarray(ids),
        int(k),
        act_path=ACT_PATH,
    )


# revision 16
# speedup vs baseline: 14.0450x; 1.8310x over previous
"""Trainium2 Bass kernel for brute-force kNN (nn_BruteForce_72541997629642).

Problem: queries [256, 64] f32, candidates [1e6, 64] f32, ids [1e6] i32,
k=10.  reference: scores = queries @ candidates.T; top_k(scores, k).

Strategy (8 NeuronCores, candidates sharded along N, queries replicated):
  Device (per core, 125k candidates, padded to 2*W):
    - host pre-transposes + bf16-casts the candidate shard into
      cT [128, W]: partition rows 0:64 hold dims of shard candidates
      [0, W) ("half A"), rows 64:128 hold dims of candidates [W, 2W)
      ("half B") -> every DMA uses all 128 partitions at full width,
    - TensorE: bf16 matmuls, queries stationary ([64, 128] per query
      group, auto row-tiled at partition base 0/64), candidates moving
      (512 cols per matmul = 1 PSUM bank),
    - VectorE: reduce_max over 32-candidate blocks straight from PSUM
      ([128, 64, 32] -> [128, 64] per 4-bank PSUM tile),
    - optional ScalarE-assisted path (ACT_FRAC of tiles): ACT casts PSUM
      fp32 -> SBUF bf16, DVE folds chunk pairs with 2x-mode bf16
      tensor-tensor max before a bf16 reduce (drains PSUM faster than the
      1x-only tensor_reduce can alone),
    - DMA the per-query block-max table bm [2, 128, NB] to DRAM.
  Host:
    - pick top BLK_TAKE block-entries per (core, query) from bm,
    - gather those blocks' candidate vectors, exact fp32 rescore,
      merge all cores, exact global top-k.

The bf16 matmul only drives block *selection* (top-k blocks by block-max
provably contain the exact top-k candidates; bf16 noise is covered by the
BLK_TAKE margin over k).  All returned scores/indices come from the exact
fp32 host rescore, so indices match the fp32 reference exactly and scores
to ~1e-7 relative.
"""

from contextlib import ExitStack

import ml_dtypes
import numpy as np

import concourse.bass as bass
import concourse.mybir as mybir
import concourse.tile as tile
from concourse.bass_utils import run_bass_kernel_spmd
from concourse.vector_clock import ScopedClock

f32 = mybir.dt.float32
bf16 = mybir.dt.bfloat16

# ---------------- problem constants (hardcoded per spec) ----------------
B = 256          # queries
D = 64           # dims
N = 1_000_000    # candidates
NCORES = 8
NSHARD = N // NCORES          # 125000 candidates per core
W = 63488                     # half-shard width (= 31 * 2048)
NPAD = 2 * W                  # padded per-core candidates (126976)
BS = 32                       # block size for block-max
NB = NPAD // BS               # block slots per core (3968)
F_DMA = 2048                  # candidate columns per DMA tile
F_MM = 512                    # moving free dim per matmul (1 PSUM bank)
N_DMA_TILES = W // F_DMA      # 31
BLK_TAKE = 16                 # block entries taken per (core, query) on host

# ACT-assisted PSUM drain: for tiles with ti % ACT_DIRECT_EVERY != 0, the
# ScalarE casts both halves' PSUM scores to SBUF bf16 and the VectorE folds
# them (2x bf16 tensor-tensor max) before one bf16 reduce — draining PSUM
# ~1.5x faster than the 1x-only fp32 tensor_reduce path can alone. The
# resulting block "entries" each cover 4 blocks (128 candidates); the host
# expands them. Tiles with ti % ACT_DIRECT_EVERY == 0 use the direct DVE
# reduce (keeps the VectorE busy while ScalarE is the unit bottleneck).
ACT_PATH = True
ACT_DIRECT_EVERY = 1000
# Decoupled ACT/DVE PSUM streams (see _build_nc split_geometry)
SPLIT_GEOMETRY = True

_MAX_WAITS = 1


def _split_excess_waits(nc):
    """Walrus codegen rejects instructions carrying more than ~1 sem-wait
    (varies by ISA struct).  Move excess waits onto same-engine NoOps
    inserted immediately before the offending instruction — the engine
    blocks at the NoOps instead, which is semantically identical."""
    n_nops = 0
    for f in nc.m.functions:
        for bb in f.blocks:
            new_insts = []
            dirty = False
            for ins in bb.instructions:
                si = ins.sync_info
                if (
                    si is not None
                    and si.on_wait is not None
                    and len(si.on_wait) > _MAX_WAITS
                ):
                    waits = list(si.on_wait)
                    keep = waits[: _MAX_WAITS]
                    rest = waits[_MAX_WAITS:]
                    for j in range(0, len(rest), _MAX_WAITS):
                        nop = mybir.InstNoOp(name=f"I-waitsplit-{n_nops}")
                        n_nops += 1
                        nop.engine = ins.engine
                        nop.sync_info = mybir.SyncInfo(
                            on_wait=rest[j : j + _MAX_WAITS], on_update=[]
                        )
                        new_insts.append(nop)
                    ins.sync_info = mybir.SyncInfo(
                        on_wait=keep, on_update=list(si.on_update or [])
                    )
                    dirty = True
                new_insts.append(ins)
            if dirty:
                bb.instructions = new_insts
    return n_nops


def _build_nc(repeat: int = 1, act_path: bool = ACT_PATH, loop_repeat: int = 0,
              psum_bufs: int = 2, cpool_bufs: int = 3, sbp_bufs: int = 4,
              cast_halves: bool = False, use_f3: bool = False,
              direct_every: int | None = None, split_geometry: bool = False):
    """Build the per-core Bass program. See module docstring for layout.

    loop_repeat > 0 wraps the streaming body in a device-side For loop that
    runs it that many times — used only for wall-clock timing of the
    steady-state kernel (amortizes dispatch/transfer overheads).
    """
    nc = bass.Bass()
    qT = nc.dram_tensor("qT", [128, B], bf16, kind="ExternalInput")
    cT = nc.dram_tensor("cT", [128, W], bf16, kind="ExternalInput")
    bm = nc.dram_tensor("bm", [2, 128, NB], f32, kind="ExternalOutput")

    with tile.TileContext(nc) as tc, ExitStack() as ctx:
        de = ACT_DIRECT_EVERY if direct_every is None else direct_every
        qpool = ctx.enter_context(tc.tile_pool(name="qpool", bufs=1))
        cpool = ctx.enter_context(tc.tile_pool(name="cpool", bufs=cpool_bufs))
        pp = ctx.enter_context(tc.tile_pool(name="pp", bufs=psum_bufs, space="PSUM"))
        bmp = ctx.enter_context(tc.tile_pool(name="bmp", bufs=1))
        sbp = ctx.enter_context(tc.tile_pool(name="sbp", bufs=sbp_bufs))

        qt = qpool.tile([128, B], bf16)
        nc.sync.dma_start(out=qt[:], in_=qT[:])
        bm_sb = [
            bmp.tile([128, NB], f32, name=f"bmsb{g}", tag=f"bmsb{g}") for g in range(2)
        ]
        if act_path:
            # ACT units leave most bm slots unwritten (host masks them);
            # memset once so the output DMA never reads uninitialized SBUF.
            for g in range(2):
                nc.gpsimd.memset(bm_sb[g][:], -3.0e38)

        from contextlib import nullcontext

        def body_ctx():
            if loop_repeat > 0:
                return tc.For_i(
                    0,
                    loop_repeat,
                    1,
                    hint_engines=(
                        mybir.EngineType.PE,
                        mybir.EngineType.DVE,
                        mybir.EngineType.SP,
                        mybir.EngineType.Activation,
                    ),
                )
            return nullcontext()

        with body_ctx():
          for _rep in range(repeat):
            for ti in range(N_DMA_TILES):
                ct = cpool.tile([128, F_DMA], bf16)
                nc.sync.dma_start(
                    out=ct[:], in_=cT[:, ti * F_DMA : (ti + 1) * F_DMA]
                )
                if split_geometry:
                    # Decoupled streams: per (g, ti) the first 3 512-subs of
                    # each half go to an ACT-drained 3-bank PSUM tile; the
                    # last sub of both halves shares a 2-bank direct tile
                    # drained by DVE reduce_max. ACT pipeline (psa bufs=2)
                    # never waits on DVE, and vice versa.
                    for g in range(2):
                        boff = [ti * 64, NB // 2 + ti * 64]
                        atiles = []
                        for h in range(2):
                            pa = pp.tile([128, 1536], f32, name="psa", tag="psa", bufs=2)
                            for sub in range(3):
                                nc.tensor.matmul(
                                    out=pa[:, 512 * sub : 512 * (sub + 1)],
                                    lhsT=qt[64 * h : 64 * (h + 1), 128 * g : 128 * (g + 1)],
                                    rhs=ct[64 * h : 64 * (h + 1), 512 * sub : 512 * (sub + 1)],
                                    start=True, stop=True,
                                )
                            cst = sbp.tile([128, 1536], bf16, name=f"csta{h}", tag=f"csta{h}")
                            nc.scalar.copy(out=cst[:], in_=pa[:])
                            atiles.append(cst)
                        pd = pp.tile([128, 1024], f32, name="psd", tag="psd", bufs=1)
                        for h in range(2):
                            nc.tensor.matmul(
                                out=pd[:, 512 * h : 512 * (h + 1)],
                                lhsT=qt[64 * h : 64 * (h + 1), 128 * g : 128 * (g + 1)],
                                rhs=ct[64 * h : 64 * (h + 1), 1536:2048],
                                start=True, stop=True,
                            )
                        # direct reduces: h-half sub3 -> slots boff[h]+48..64
                        for h in range(2):
                            nc.vector.reduce_max(
                                out=bm_sb[g][:, boff[h] + 48 : boff[h] + 64],
                                in_=pd[:, 512 * h : 512 * (h + 1)].rearrange(
                                    "p (nb bs) -> p nb bs", bs=BS
                                ),
                                axis=mybir.AxisListType.X,
                            )
                        # act fold: f1 = max(A0, A1); f2 = halves; red -> 24
                        # entries at slots boff[0]..boff[0]+24
                        f1 = sbp.tile([128, 1536], bf16, name="f1s", tag="f1s")
                        nc.vector.tensor_tensor(
                            out=f1[:], in0=atiles[0][:], in1=atiles[1][:],
                            op=mybir.AluOpType.max,
                        )
                        f2 = sbp.tile([128, 768], bf16, name="f2s", tag="f2s")
                        nc.vector.tensor_tensor(
                            out=f2[:], in0=f1[:, :768], in1=f1[:, 768:],
                            op=mybir.AluOpType.max,
                        )
                        nc.vector.reduce_max(
                            out=bm_sb[g][:, boff[0] : boff[0] + 24],
                            in_=f2[:].rearrange("p (nb bs) -> p nb bs", bs=BS),
                            axis=mybir.AxisListType.X,
                        )
                    continue
                use_act = act_path and (ti % de != 0)
                for g in range(2):
                    boff_a = (ti * F_DMA) // BS          # half A slots
                    if not use_act:
                        for h in range(2):
                            boff = (h * W + ti * F_DMA) // BS
                            ps = pp.tile([128, F_DMA], f32, name="ps", tag="ps")
                            for sub in range(4):
                                nc.tensor.matmul(
                                    out=ps[:, 512 * sub : 512 * (sub + 1)],
                                    lhsT=qt[
                                        64 * h : 64 * (h + 1), 128 * g : 128 * (g + 1)
                                    ],
                                    rhs=ct[
                                        64 * h : 64 * (h + 1),
                                        512 * sub : 512 * (sub + 1),
                                    ],
                                    start=True,
                                    stop=True,
                                )
                            nc.vector.reduce_max(
                                out=bm_sb[g][:, boff : boff + F_DMA // BS],
                                in_=ps[:].rearrange("p (nb bs) -> p nb bs", bs=BS),
                                axis=mybir.AxisListType.X,
                            )
                    else:
                        # ACT unit: both halves' scores (4096 candidates).
                        # ACT casts each PSUM mega-tile to SBUF bf16; DVE:
                        # f1 = max(castA, castB)  [128, 2048]  (bf16 2x)
                        # f2 = max(f1[:, :1024], f1[:, 1024:])
                        # red = blockmax(f2)      [128, 32]
                        # entry b covers blocks {A: b, b+32, B: b, b+32}.
                        casts = []
                        for h in range(2):
                            ps = pp.tile([128, F_DMA], f32, name="ps", tag="ps")
                            for sub in range(4):
                                nc.tensor.matmul(
                                    out=ps[:, 512 * sub : 512 * (sub + 1)],
                                    lhsT=qt[
                                        64 * h : 64 * (h + 1), 128 * g : 128 * (g + 1)
                                    ],
                                    rhs=ct[
                                        64 * h : 64 * (h + 1),
                                        512 * sub : 512 * (sub + 1),
                                    ],
                                    start=True,
                                    stop=True,
                                )
                            cst = sbp.tile(
                                [128, F_DMA], bf16, name=f"cast{h}", tag=f"cast{h}"
                            )
                            if cast_halves:
                                nc.scalar.copy(out=cst[:, : F_DMA // 2], in_=ps[:, : F_DMA // 2])
                                nc.scalar.copy(out=cst[:, F_DMA // 2 :], in_=ps[:, F_DMA // 2 :])
                            else:
                                nc.scalar.copy(out=cst[:], in_=ps[:])
                            casts.append(cst)
                        f1 = sbp.tile([128, F_DMA], bf16, name="f1", tag="f1")
                        nc.vector.tensor_tensor(
                            out=f1[:], in0=casts[0][:], in1=casts[1][:],
                            op=mybir.AluOpType.max,
                        )
                        f2 = sbp.tile([128, F_DMA // 2], bf16, name="f2", tag="f2")
                        nc.vector.tensor_tensor(
                            out=f2[:], in0=f1[:, : F_DMA // 2], in1=f1[:, F_DMA // 2 :],
                            op=mybir.AluOpType.max,
                        )
                        if use_f3:
                            f3 = sbp.tile([128, F_DMA // 4], bf16, name="f3", tag="f3")
                            nc.vector.tensor_tensor(
                                out=f3[:], in0=f2[:, : F_DMA // 4],
                                in1=f2[:, F_DMA // 4 :], op=mybir.AluOpType.max,
                            )
                            nc.vector.reduce_max(
                                out=bm_sb[g][:, boff_a : boff_a + 16],
                                in_=f3[:].rearrange("p (nb bs) -> p nb bs", bs=BS),
                                axis=mybir.AxisListType.X,
                            )
                        else:
                            nc.vector.reduce_max(
                                out=bm_sb[g][:, boff_a : boff_a + 32],
                                in_=f2[:].rearrange("p (nb bs) -> p nb bs", bs=BS),
                                axis=mybir.AxisListType.X,
                            )
            for g in range(2):
                nc.sync.dma_start(out=bm[g], in_=bm_sb[g][:])
    _split_excess_waits(nc)
    nc.finalize()
    return nc


_NC_CACHE: dict[tuple, object] = {}


def get_nc(repeat: int = 1, act_path: bool = ACT_PATH):
    key = (repeat, act_path, SPLIT_GEOMETRY)
    if key not in _NC_CACHE:
        _NC_CACHE[key] = _build_nc(repeat, act_path, split_geometry=SPLIT_GEOMETRY)
    return _NC_CACHE[key]


def _prep_inputs(queries: np.ndarray, candidates: np.ndarray):
    q = np.asarray(queries, dtype=np.float32)
    c = np.asarray(candidates, dtype=np.float32)
    qT = np.ascontiguousarray(q.T)  # [64, 256]
    qT2 = np.concatenate([qT, qT], axis=0).astype(ml_dtypes.bfloat16)
    in_maps = []
    for core in range(NCORES):
        shard = c[core * NSHARD : (core + 1) * NSHARD]
        half_a = shard[:W]
        half_b = shard[W:]
        cT2 = np.zeros((128, W), dtype=ml_dtypes.bfloat16)
        cT2[:D, :] = half_a.T.astype(ml_dtypes.bfloat16)
        cT2[D:, : half_b.shape[0]] = half_b.T.astype(ml_dtypes.bfloat16)
        in_maps.append({"qT": qT2, "cT": cT2})
    return in_maps


def _host_finish(bm_all, queries, candidates, ids, k, act_path: bool = ACT_PATH):
    """bm_all: [NCORES, 2, 128, NB] f32 block-max tables -> exact top-k."""
    q = np.asarray(queries, dtype=np.float32)
    c = np.asarray(candidates, dtype=np.float32)
    ids = np.asarray(ids)
    k = int(k)
    bm = bm_all.reshape(NCORES, B, NB).transpose(1, 0, 2).copy()  # [B, 8, NB]

    NBH = W // BS  # slots per half
    if act_path and SPLIT_GEOMETRY:
        # per tile of 64 slots (per half): h0 offsets 0-23 = ACT entries
        # (each covers blocks {s, s+24, s+NBH, s+NBH+24}), 24-47 dead,
        # 48-63 direct; h1 offsets 0-47 dead, 48-63 direct.
        slots = np.arange(NB)
        in_half_a = slots < NBH
        off = np.where(in_half_a, slots % 64, (slots - NBH) % 64)
        entry_slot = in_half_a & (off < 24)
        direct_slot = off >= 48
        dead = ~(entry_slot | direct_slot)
        bm[:, :, dead] = -np.inf

        take = min(BLK_TAKE, NB)
        part = np.argpartition(-bm, take - 1, axis=2)[:, :, :take]
        sel_entry = entry_slot[part]
        partners = np.stack(
            [part, part + 24, part + NBH, part + NBH + 24], axis=-1
        )
        blocks = np.where(sel_entry[..., None], partners, part[..., None])
        blocks = blocks.reshape(B, NCORES, -1)
    elif act_path:
        # ACT units only write the first 32 slots of their tile's half-A
        # range; every other slot of an ACT tile is dead (-3e38 from the
        # device memset). Mask defensively, then expand selected entry
        # slots to their 4 covered blocks.
        slots = np.arange(NB)
        in_half_a = slots < NBH
        ti_of = np.where(in_half_a, slots // 64, (slots - NBH) // 64)
        is_act = (ti_of % ACT_DIRECT_EVERY) != 0
        b_in_tile = np.where(in_half_a, slots % 64, (slots - NBH) % 64)
        entry_slot = in_half_a & is_act & (b_in_tile < 32)
        dead = is_act & ~entry_slot
        bm[:, :, dead] = -np.inf

        take = min(BLK_TAKE, NB)
        part = np.argpartition(-bm, take - 1, axis=2)[:, :, :take]  # [B, 8, T]
        sel_entry = entry_slot[part]  # [B, 8, T]
        partners = np.stack(
            [part, part + 32, part + NBH, part + NBH + 32], axis=-1
        )  # [B, 8, T, 4]
        blocks = np.where(sel_entry[..., None], partners, part[..., None])
        blocks = blocks.reshape(B, NCORES, -1)
    else:
        take = min(BLK_TAKE, NB)
        part = np.argpartition(-bm, take - 1, axis=2)[:, :, :take]
        blocks = part

    local = blocks[..., None] * BS + np.arange(BS)  # [B, 8, T', 32]
    core_off = (np.arange(NCORES) * NSHARD)[None, :, None, None]
    valid = local < NSHARD
    gidx = np.where(valid, local + core_off, 0)
    Bq = B
    gidx = gidx.reshape(Bq, -1)
    valid = valid.reshape(Bq, -1)

    P = gidx.shape[1]
    top_scores = np.empty((B, k), dtype=np.float32)
    top_idx = np.empty((B, k), dtype=np.int32)
    CH = 64
    for q0 in range(0, B, CH):
        q1 = min(q0 + CH, B)
        gi = gidx[q0:q1]
        gath = c[gi]  # [ch, P, 64]
        sc = np.einsum("qd,qpd->qp", q[q0:q1], gath, optimize=True)
        sc = np.where(valid[q0:q1], sc, -np.inf).astype(np.float32)
        for qq in range(q1 - q0):
            row = sc[qq]
            gx = gi[qq]
            m = min(4 * k, P - 1)
            sel = np.argpartition(-row, m)[: m + 1]
            order = np.lexsort((gx[sel], -row[sel]))
            seen = set()
            out_s, out_i = [], []
            for o in order:
                cid = int(gx[sel[o]])
                s = row[sel[o]]
                if cid in seen or not np.isfinite(s):
                    continue
                seen.add(cid)
                out_s.append(s)
                out_i.append(cid)
                if len(out_s) == k:
                    break
            top_scores[q0 + qq] = out_s
            top_idx[q0 + qq] = ids[np.asarray(out_i, dtype=np.int64)]
    return top_scores, top_idx


def kernel(queries, candidates, ids, k):
    k = int(k)
    assert 1 <= k <= BLK_TAKE, (
        f"kernel compiled for k <= {BLK_TAKE} (top-k-block selection margin); got {k}"
    )
    in_maps = _prep_inputs(queries, candidates)
    nc = get_nc(repeat=1, act_path=ACT_PATH)
    res = run_bass_kernel_spmd(nc, in_maps, core_ids=list(range(NCORES)))
    bm_all = np.stack([res.results[c]["bm"] for c in range(NCORES)])
    return _host_finish(
        bm_all,
        np.asarray(queries, np.float32),
        np.asarray(candidates, np.float32),
        np.asarray(ids),
        int(k),
        act_path=ACT_PATH,
    )
